# revision 1
# baseline (speedup 1.0000x reference)
"""Trainium2 Bass kernel: Chebyshev graph filter (8-core SPMD).

kernel(rows, cols, vals, X) -> [N, 64] float32, matching
  acc = sum_k c_k T_k(L - I) X, L = COO(rows, cols, vals), M=30 terms
(terms with |tail| < 1e-7 truncated; exact to f32 precision).

Design: rows sharded over 8 NeuronCores (degree-sorted, padded ELL);
per iteration each core gathers neighbor rows of the current T_{k-1}
with the MoE dma_gather instruction (int16 indices, 4 HBM source banks,
4 parallel SWDGE queues), reduces on DVE, applies the Chebyshev
recurrence via fused scalar_tensor_tensor ops, and a jax-level
all_gather rebuilds the replicated T_k between bass_jit custom calls.
"""
import sys as _sys
for _p in ("/opt/trn_rl_repo",):
    if _p not in _sys.path:
        _sys.path.insert(0, _p)
"""Chebyshev graph filter on TRN2: jax-orchestrated, dma_gather-based.

Per iteration (one bass_jit NEFF per core, jax all_gather between):
  t = mul * (sum of gathered neighbor rows) - y2 ;  acc += c_k * t
Gather: MoE dma_gather (int16 idx, 4 source banks via AP offset,
position-wrapped output), padded-ELL per (chunk, bank), degree-sorted rows.

Gather-space layout: row (shard s, partition p, group g) at flat index
s*Rpad + p*Gtot1 + g, where Gtot1 = Gtot + 1; group Gtot is all zeros
(ELL padding target), maintained by each call writing zeros there.
"""
import numpy as np
import functools

import jax
import jax.numpy as jnp
from jax.sharding import Mesh, PartitionSpec as Pspec
from jax.experimental.shard_map import shard_map

import concourse.bass as bass
import concourse.bacc as bacc
import concourse.mybir as mybir
from concourse.tile import TileContext
from concourse.bass2jax import bass_jit

P = 128
D = 64
BANK_ROWS_DEFAULT = 25344   # <= 32767 and = 2*12672


def cheb_coeffs(m=30, t_scale=5.0, lambda_max=2.0):
    j = np.arange(m, dtype=np.float64)
    x = np.cos(np.pi * (j + 0.5) / m)
    lam = lambda_max / 2.0 * (x + 1.0)
    f = np.exp(-t_scale * lam)
    ks = np.arange(m, dtype=np.float64)[:, None]
    T = np.cos(ks * np.arccos(x)[None, :])
    c = 2.0 / m * np.sum(f[None, :] * T, axis=1)
    c[0] /= 2.0
    return c


def pick_n_terms(c, abs_tol=1e-7):
    tail = np.cumsum(np.abs(c[::-1]))[::-1]
    for K in range(1, len(c) + 1):
        if K == len(c) or tail[K] <= abs_tol:
            return K
    return len(c)


def wrap_idx_positions(flat):
    """flat int list (len n, multiple of 16) -> [128, n//16] int16 wrapped in 16
    partitions, replicated 8x (one copy per Q7 core pair group)."""
    n = len(flat)
    w = np.asarray(flat, np.int16).reshape(n // 16, 16).T  # [16, n//16]
    return np.tile(w, (8, 1))


def preprocess(rows, cols, vals, n_cores=8, g_chunk=2, n_terms=None,
               bank_rows=BANK_ROWS_DEFAULT):
    rows = np.asarray(rows); cols = np.asarray(cols); vals = np.asarray(vals)
    nnz = rows.shape[0]
    N_guess = int(max(rows.max(), cols.max())) + 1
    E_guess = nnz - N_guess
    const_mode = False
    if E_guess > 0:
        ar = np.arange(N_guess, dtype=rows.dtype)
        if (rows[E_guess:] == ar).all() and (cols[E_guess:] == ar).all() \
                and (vals[E_guess:] == 1.0).all() \
                and (vals[:E_guess] == vals[0]).all():
            const_mode = True
    N = N_guess
    if const_mode:
        e_rows, e_cols = rows[:E_guess], cols[:E_guess]
        a_const = float(vals[0])
        e_wts = None
    else:
        e_rows = np.concatenate([rows, np.arange(N, dtype=rows.dtype)])
        e_cols = np.concatenate([cols, np.arange(N, dtype=cols.dtype)])
        e_wts = np.concatenate([vals.astype(np.float32),
                                np.full(N, -1.0, np.float32)])
        a_const = None

    assert N % n_cores == 0
    Rs = N // n_cores
    Gtot = (Rs + P - 1) // P          # data groups
    Gtot1 = Gtot + 1                  # + zero group
    Rpad = Gtot1 * P                  # rows per shard in gather space
    NG = n_cores * Rpad               # gather-space rows
    if bank_rows % Rpad != 0:
        bank_rows = max(1, bank_rows // Rpad) * Rpad
    n_banks = (NG + bank_rows - 1) // bank_rows
    assert bank_rows <= 32767
    assert bank_rows % Rpad == 0

    deg = np.bincount(e_rows, minlength=N)
    pi = np.empty(N, dtype=np.int64)
    for s in range(n_cores):
        lo, hi = s * Rs, (s + 1) * Rs
        order = np.argsort(-deg[lo:hi], kind="stable")
        pi[lo:hi] = lo + order
    pi_inv = np.empty(N, dtype=np.int64)
    pi_inv[pi] = np.arange(N)

    # gather-space index of new-space row id
    def gidx_of_new(new_ids):
        s = new_ids // Rs
        r = new_ids % Rs
        return s * Rpad + (r % P) * Gtot1 + (r // P)

    # CSR by destination new-row, with source gather idx + bank
    dest_new = pi_inv[e_rows]
    order = np.argsort(dest_new, kind="stable")
    dest_sorted = dest_new[order]
    src_g = gidx_of_new(pi_inv[e_cols[order]])
    src_bank = src_g // bank_rows
    src_loc = src_g - src_bank * bank_rows
    wts_sorted = e_wts[order].astype(np.float32) if e_wts is not None else None
    row_ptr = np.zeros(N + 1, dtype=np.int64)
    np.cumsum(np.bincount(dest_sorted, minlength=N), out=row_ptr[1:])

    # per (row, bank) counts
    cnt_rb = np.zeros((N, n_banks), dtype=np.int32)
    for b in range(n_banks):
        cnt_rb[:, b] = np.bincount(dest_sorted[src_bank == b], minlength=N)

    # chunk geometry: chunks of g_chunk groups of 128 rows; widths per bank,
    # max across cores (ranks r in [c*g*P, ...), any shard)
    n_chunks = (Gtot + g_chunk - 1) // g_chunk
    chunk_G = []
    chunk_W = []          # list of [n_banks] widths
    cnt_mat = np.zeros((n_cores, Gtot * P, n_banks), np.int32)
    for s in range(n_cores):
        cnt_mat[s, :Rs] = cnt_rb[s * Rs:(s + 1) * Rs]
    for ci in range(n_chunks):
        g0, g1 = ci * g_chunk, min((ci + 1) * g_chunk, Gtot)
        Wb = cnt_mat[:, g0 * P:g1 * P].max(axis=(0, 1))
        Wb = np.maximum(Wb, 1)
        chunk_G.append(g1 - g0)
        chunk_W.append([int(x) for x in Wb])

    # per-core idx streams: for each (chunk, bank): positions i = slot*128+p,
    # slot = g_local*W + w ; value = src_loc or pad (zero row of a shard
    # inside this bank: shard 2b's group Gtot -> local idx p*Gtot1 + Gtot...
    # use row (p=0,g=Gtot) of first shard in bank: (2b shard) local =
    # (0*Gtot1 + Gtot) = Gtot  (bank-local, since shard base = bank base)
    shards_per_bank = bank_rows // Rpad
    idx_streams = []     # per core: concatenated int16-wrapped arrays
    wts_streams = [] if e_wts is not None else None
    seg_meta = []        # per (chunk, bank): (G, W, n_positions)
    for s in range(n_cores):
        parts = []
        wparts = []
        for ci in range(n_chunks):
            G = chunk_G[ci]
            g0 = ci * g_chunk
            for b in range(n_banks):
                W = chunk_W[ci][b]
                pad_local = Gtot   # row (p=0, g=Gtot) of first shard in bank
                ell = np.full((P, G, W), pad_local, dtype=np.int32)
                well = np.zeros((P, G, W), dtype=np.float32) if e_wts is not None else None
                for gl in range(G):
                    g = g0 + gl
                    ranks = g * P + np.arange(P)
                    valid = ranks < Rs
                    new_ids = s * Rs + np.minimum(ranks, Rs - 1)
                    st = row_ptr[new_ids]
                    en = row_ptr[new_ids + 1]
                    # positions of bank-b edges within [st, en)
                    for p in range(P):
                        if not valid[p]:
                            continue
                        sl = slice(st[p], en[p])
                        m = src_bank[sl] == b
                        v = src_loc[sl][m]
                        ell[p, gl, :len(v)] = v
                        if well is not None:
                            well[p, gl, :len(v)] = wts_sorted[sl][m]
                # flatten to position order: i = (g*W+w)*128 + p
                flat = ell.transpose(1, 2, 0).reshape(-1)   # (g, w, p)
                parts.append(wrap_idx_positions(flat))
                if well is not None:
                    wparts.append(well.transpose(1, 2, 0).reshape(-1))
                if s == 0:
                    seg_meta.append((G, W, len(flat)))
        idx_streams.append(np.concatenate(parts, axis=1))
        if wts_streams is not None:
            wts_streams.append(np.concatenate(wparts))

    c = cheb_coeffs()
    K = n_terms if n_terms is not None else pick_n_terms(c)
    return dict(
        N=N, n_cores=n_cores, Rs=Rs, Rpad=Rpad, Gtot=Gtot, Gtot1=Gtot1,
        NG=NG, n_banks=n_banks, bank_rows=bank_rows,
        n_chunks=n_chunks, chunk_G=chunk_G, chunk_W=chunk_W,
        seg_meta=seg_meta, const_mode=const_mode, a_const=a_const,
        pi=pi, pi_inv=pi_inv, coeffs=c, K=K, g_chunk=g_chunk,
        idx_streams=idx_streams, wts_streams=wts_streams,
    )


def build_x_inputs(X, meta):
    n_cores, Rs, Gtot, Gtot1 = (meta[k] for k in
                                ("n_cores", "Rs", "Gtot", "Gtot1"))
    N = meta["N"]; pi = meta["pi"]
    d = X.shape[1]
    xsh = []       # [P, Gtot*d] per core (data groups only)
    x0_blocks = []
    for s in range(n_cores):
        Xs = np.zeros((Gtot1 * P, d), dtype=np.float32)
        Xs[:Rs] = X[pi[s * Rs:(s + 1) * Rs]]
        Xs[Rs:] = 0.0
        # rank r = g*P + p -> partition-major [P, Gtot1, d]
        Xpm = Xs.reshape(Gtot1, P, d).transpose(1, 0, 2)
        xsh.append(np.ascontiguousarray(Xpm[:, :Gtot].reshape(P, Gtot * d)))
        x0_blocks.append(Xpm.reshape(P * Gtot1, d))
    x0 = np.concatenate(x0_blocks, axis=0)  # [NG, d]
    return xsh, np.ascontiguousarray(x0)


def make_step_fn(meta, d=D):
    """Returns bass_jit'd per-core step: (ysrc, y2, acc, idx[, wts], consts)
    -> (t_out [P, Gtot1*d], acc_out)."""
    n_chunks = meta["n_chunks"]
    n_banks = meta["n_banks"]
    chunk_G = meta["chunk_G"]; chunk_W = meta["chunk_W"]
    g_chunk = meta["g_chunk"]
    Gtot, Gtot1, NG = meta["Gtot"], meta["Gtot1"], meta["NG"]
    bank_rows = meta["bank_rows"]
    const_mode = meta["const_mode"]
    f32 = mybir.dt.float32
    i16 = mybir.dt.int16

    def step(nc, ysrc, y2, acc_in, idx, wts, consts):
        with TileContext(nc) as tc:
            with (
                tc.tile_pool(name="state", bufs=1) as st,
                tc.tile_pool(name="gpool", bufs=4) as gp,
                tc.tile_pool(name="ipool", bufs=4) as ip,
                tc.tile_pool(name="wpool", bufs=3) as wp,
            ):
                y2sb = st.tile([P, Gtot * d], f32, name="y2sb")
                nc.sync.dma_start(out=y2sb[:], in_=y2.ap())
                accsb = st.tile([P, Gtot * d], f32, name="accsb")
                nc.sync.dma_start(out=accsb[:], in_=acc_in.ap())
                csb = st.tile([P, 2], f32, name="csb")
                nc.sync.dma_start(out=csb[:], in_=consts.ap())
                zt = st.tile([P, d], f32, name="zt")
                nc.vector.memset(zt[:], 0.0)

                t_out = nc.dram_tensor("t_out", [P, Gtot1 * d], f32,
                                       kind="ExternalOutput")
                acc_out = nc.dram_tensor("acc_out", [P, Gtot * d], f32,
                                         kind="ExternalOutput")
                # zero group of t_out
                nc.sync.dma_start(out=t_out.ap()[:, Gtot * d:], in_=zt[:])

                seg_off = 0      # idx stream offset (int16 columns)
                woff = 0         # wts stream offset (f32 per-position)
                si = 0
                for ci in range(n_chunks):
                    G = chunk_G[ci]
                    g0 = ci * g_chunk
                    reds = []
                    for b in range(n_banks):
                        W = chunk_W[ci][b]
                        npos = P * G * W
                        ncols = npos // 16
                        it = ip.tile([P, max(16, ncols)], i16, tag="it", name="it")
                        nc.sync.dma_start(out=it[:, :ncols],
                                          in_=idx.ap()[:, seg_off:seg_off + ncols])
                        gb = gp.tile([P, G * W * d], f32, tag="gb", name="gb")
                        src = ysrc.ap()[b * bank_rows:
                                        min((b + 1) * bank_rows, NG), :]
                        nc.gpsimd.dma_gather(
                            out_ap=gb[:].rearrange("p (s f) -> p s f", f=d),
                            in_ap=src,
                            idxs_ap=it[:, :ncols],
                            num_idxs=npos,
                            num_idxs_reg=npos,
                            elem_size=d,
                            single_packet=False,
                            queue_num=b % 4,
                        )
                        if not const_mode:
                            wt = ip.tile([P, G * W], f32, tag="wt", name="wt")
                            nc.sync.dma_start(
                                out=wt[:],
                                in_=wts.ap()[:, woff:woff + G * W])
                            wview = bass.AP(
                                wt.tensor, wt[:].offset,
                                [wt[:].ap[0], [1, G * W], [0, d]])
                            nc.vector.tensor_tensor(
                                out=gb[:].rearrange("p (s f) -> p s f", f=d),
                                in0=gb[:].rearrange("p (s f) -> p s f", f=d),
                                in1=wview, op=mybir.AluOpType.mult)
                            woff += G * W
                        red = wp.tile([P, G * d], f32, tag=f"red{b}",
                                      name=f"red{b}")
                        nc.vector.tensor_reduce(
                            out=red[:].rearrange("p (g f) -> p g f", f=d),
                            in_=gb[:].rearrange("p (g w f) -> p g f w",
                                                g=G, w=W, f=d),
                            axis=mybir.AxisListType.X,
                            op=mybir.AluOpType.add)
                        reds.append(red[:])
                        seg_off += ncols
                        si += 1
                    # combine banks
                    stot = reds[0]
                    for b in range(1, n_banks):
                        nc.vector.tensor_tensor(
                            out=stot, in0=stot, in1=reds[b],
                            op=mybir.AluOpType.add)
                    seg = slice(g0 * d, (g0 + G) * d)
                    tt = wp.tile([P, G * d], f32, tag="tt", name="tt")
                    # t = mul*s - y2   (mul = consts[:,0:1] per-partition)
                    nc.vector.scalar_tensor_tensor(
                        out=tt[:], in0=stot, scalar=csb[:, 0:1],
                        in1=y2sb[:, seg], op0=mybir.AluOpType.mult,
                        op1=mybir.AluOpType.subtract)
                    # acc += c_k * t
                    nc.vector.scalar_tensor_tensor(
                        out=accsb[:, seg], in0=tt[:], scalar=csb[:, 1:2],
                        in1=accsb[:, seg], op0=mybir.AluOpType.mult,
                        op1=mybir.AluOpType.add)
                    nc.sync.dma_start(out=t_out.ap()[:, seg], in_=tt[:])
                nc.sync.dma_start(out=acc_out.ap(), in_=accsb[:])
        nc.compile()
        return t_out, acc_out

    nq = min(4, n_banks) if n_banks > 1 else 1
    if meta["const_mode"]:
        def step_c(nc, ysrc, y2, acc_in, idx, consts):
            return step(nc, ysrc, y2, acc_in, idx, None, consts)
        return bass_jit(step_c, num_swdge_queues=nq)
    return bass_jit(step, num_swdge_queues=nq)


def make_driver(meta, d=D):
    """Two jits: step (pure bass custom call) + exchange (all_gather)."""
    n_cores = meta["n_cores"]
    Gtot, Gtot1, NG = meta["Gtot"], meta["Gtot1"], meta["NG"]
    step_fn = make_step_fn(meta)
    devs = jax.devices()[:n_cores]
    mesh = Mesh(np.array(devs), ("core",))
    Pc = Pspec("core")
    if meta["const_mode"]:
        in_specs = (Pspec(), Pc, Pc, Pc, Pspec())
    else:
        in_specs = (Pspec(), Pc, Pc, Pc, Pc, Pspec())
    step_sm = jax.jit(shard_map(step_fn, mesh=mesh, in_specs=in_specs,
                                out_specs=(Pc, Pc), check_rep=False),
                      donate_argnums=(2,))

    def exch(t):
        yf = jax.lax.all_gather(t, "core").reshape(NG, d)
        return yf, t[:, :Gtot * d]
    exch_sm = jax.jit(shard_map(exch, mesh=mesh, in_specs=(Pc,),
                                out_specs=(Pspec(), Pc), check_rep=False))
    return step_sm, exch_sm


def run_cheb(rows, cols, vals, X, g_chunk=2, n_terms=None, prebuilt=None):
    if prebuilt is None:
        meta = preprocess(rows, cols, vals, n_cores=8, g_chunk=g_chunk,
                          n_terms=n_terms)
        step_sm, exch_sm = make_driver(meta)
    else:
        meta, step_sm, exch_sm = prebuilt
    n_cores = meta["n_cores"]
    Gtot, Gtot1 = meta["Gtot"], meta["Gtot1"]
    c = meta["coeffs"]; K = meta["K"]
    a = meta["a_const"] if meta["const_mode"] else 1.0
    xsh, x0 = build_x_inputs(X, meta)

    from jax.sharding import NamedSharding
    devs = jax.devices()[:n_cores]
    mesh = Mesh(np.array(devs), ("core",))
    shard_c = NamedSharding(mesh, Pspec("core"))
    shard_r = NamedSharding(mesh, Pspec())
    idx_stk = np.concatenate(meta["idx_streams"], axis=0)  # [8*128, COLS]
    idx_stk = jax.device_put(idx_stk, shard_c)
    wts_stk = None
    if not meta["const_mode"]:
        wparts = []
        for s in range(n_cores):
            flat = meta["wts_streams"][s]
            out_l, off = [], 0
            for (G, W, npos) in meta["seg_meta"]:
                seg = flat[off:off + npos].reshape(G, W, P).transpose(2, 0, 1)
                out_l.append(seg.reshape(P, G * W))
                off += npos
            wparts.append(np.concatenate(out_l, axis=1))
        wts_stk = jax.device_put(np.concatenate(wparts, axis=0), shard_c)
    xsh_stk = np.concatenate(xsh, axis=0)                  # [8*128, Gtot*d]
    acc = jax.device_put((c[0] * xsh_stk).astype(np.float32), shard_c)
    yfull = jax.device_put(x0, shard_r)
    y2 = jax.device_put(np.zeros_like(xsh_stk), shard_c)
    y2_next = jax.device_put(xsh_stk, shard_c)

    consts = []
    for k in range(1, K):
        cs = np.zeros((P, 2), np.float32)
        cs[:, 0] = a if k == 1 else 2 * a
        cs[:, 1] = c[k]
        consts.append(jax.device_put(cs, shard_r))

    for k in range(1, K):
        if meta["const_mode"]:
            t, acc = step_sm(yfull, y2, acc, idx_stk, consts[k - 1])
        else:
            t, acc = step_sm(yfull, y2, acc, idx_stk, wts_stk, consts[k - 1])
        if k != K - 1:
            yfull, tsl = exch_sm(t)
            y2 = y2_next
            y2_next = tsl
    accf = np.asarray(acc)

    N, Rs = meta["N"], meta["Rs"]
    out = np.empty((N, D), dtype=np.float32)
    for s in range(n_cores):
        res = accf[s * P:(s + 1) * P].reshape(P, Gtot, D)
        res = res.transpose(1, 0, 2).reshape(Gtot * P, D)
        out[meta["pi"][s * Rs:(s + 1) * Rs]] = res[:Rs]
    return out, meta


# ---------------------------------------------------------------------------
# Harness entry point
# ---------------------------------------------------------------------------
_CACHE = {}


def kernel(rows, cols, vals, X):
    """Full-input entry: distributes across 8 NeuronCores internally."""
    rows = np.asarray(rows); cols = np.asarray(cols)
    vals = np.asarray(vals); X = np.asarray(X, np.float32)
    key = (rows.tobytes(), cols.tobytes(), vals.tobytes())
    if key not in _CACHE:
        meta = preprocess(rows, cols, vals, n_cores=8, g_chunk=2)
        step_sm, exch_sm = make_driver(meta)
        _CACHE.clear()
        _CACHE[key] = (meta, step_sm, exch_sm)
    meta, step_sm, exch_sm = _CACHE[key]
    out, _ = run_cheb(rows, cols, vals, X, prebuilt=(meta, step_sm, exch_sm))
    return out.astype(np.float32)

